# revision 4
# baseline (speedup 1.0000x reference)
"""Trainium2 Bass kernel for nn_FFTChainMatrix (block-circulant matmul via 64-pt rFFT).

y = x @ W.T where W is 4096x4096 block-circulant (64x64 grid of 64x64 circulant
blocks) built from channel-weighted circulant_params.  Computed in the FFT
domain as three 128x128-matmul stages per 512-token shard:

  T_in   PE-transpose x (tok-major) -> feature-major
  S1     rfft along block dim:      X1 = A_bd.T @ xt      (per 128-feat chunk)
  shuf   partition shuffle i-pair-major -> freq-pair-major (SBUF->SBUF DMA)
  S2     per-freq complex multiply+sum over blocks: Y2 = G[fp].T @ X2
  unshuf inverse shuffle -> o-pair-major
  S3     irfft:                     Y4 = B_bd.T @ Y3
  T_out  PE-transpose back to tok-major, DMA out

Sharding: data-parallel over tokens, 4096 tokens -> 8 cores x 512.
Matmul stages run as float32r (full-rate fp32 path); transposes exact fp32.
"""

import math
from contextlib import ExitStack

import numpy as np

BLK = 64
NB = 64           # blocks per side
T = 512           # tokens per core
NCORES = 8
FEAT = 4096

MM_DT = "f32r"    # "f32r" (fast) or "f32" (exact, 4x slower stages)


# ---------------------------------------------------------------- host math
def _build_matrices(circulant_params, channel_weights):
    """A_bd (128,128), G (32,128,128), B_bd (128,128) float32, exact f64 math."""
    c_w = np.einsum(
        "m,moid->oid",
        np.asarray(channel_weights, np.float64),
        np.asarray(circulant_params, np.float64),
    )
    Chat = np.fft.rfft(c_w, axis=-1)
    Wr, Wi = Chat.real, Chat.imag

    r = np.arange(BLK)
    A64 = np.zeros((BLK, BLK))
    A64[0, :] = 1.0
    A64[1, :] = (-1.0) ** r
    B64 = np.zeros((BLK, BLK))
    B64[:, 0] = 1.0 / BLK
    B64[:, 1] = ((-1.0) ** r) / BLK
    for p in range(1, 32):
        cc = np.cos(2 * np.pi * p * r / BLK)
        ss = np.sin(2 * np.pi * p * r / BLK)
        A64[2 * p, :] = cc
        A64[2 * p + 1, :] = -ss
        B64[:, 2 * p] = 2.0 * cc / BLK
        B64[:, 2 * p + 1] = -2.0 * ss / BLK

    A_bd = np.zeros((128, 128))
    for b in range(2):
        for fp in range(32):
            for c1 in range(2):
                A_bd[64 * b: 64 * b + 64, 64 * c1 + 2 * fp + b] = A64[2 * fp + c1, :]

    iperm = 32 * (np.arange(NB) % 2) + np.arange(NB) // 2
    G = np.zeros((32, 128, 128))
    for fp in range(32):
        if fp == 0:
            for i in range(NB):
                G[0, iperm[i], iperm] = Wr[:, i, 0]
                G[0, 64 + iperm[i], 64 + iperm] = Wr[:, i, 32]
        else:
            for i in range(NB):
                G[fp, iperm[i], iperm] = Wr[:, i, fp]
                G[fp, 64 + iperm[i], iperm] = -Wi[:, i, fp]
                G[fp, iperm[i], 64 + iperm] = Wi[:, i, fp]
                G[fp, 64 + iperm[i], 64 + iperm] = Wr[:, i, fp]

    B_bd = np.zeros((128, 128))
    for b in range(2):
        B_bd[64 * b: 64 * b + 64, 64 * b: 64 * b + 64] = B64.T

    return (A_bd.astype(np.float32), G.astype(np.float32), B_bd.astype(np.float32))


# ---------------------------------------------------------------- bass trace
def _trace_nc():
    import concourse.mybir as mybir
    import concourse.tile as tile
    from concourse import bacc
    from concourse.bass import ts
    from concourse.masks import make_identity

    f32 = mybir.dt.float32
    mm_dt = mybir.dt.float32r if MM_DT == "f32r" else mybir.dt.float32

    nc = bacc.Bacc("TRN2", target_bir_lowering=False, debug=False,
                   num_devices=NCORES)
    x_h = nc.dram_tensor("x_shard", [T, FEAT], f32, kind="ExternalInput").ap()
    a_h = nc.dram_tensor("a_bd", [128, 128], mm_dt, kind="ExternalInput").ap()
    g_h = nc.dram_tensor("g_mats", [128, 32 * 128], mm_dt,
                         kind="ExternalInput").ap()
    b_h = nc.dram_tensor("b_bd", [128, 128], mm_dt, kind="ExternalInput").ap()
    y_h = nc.dram_tensor("y_shard", [T, FEAT], f32, kind="ExternalOutput").ap()

    with tile.TileContext(nc) as tc, ExitStack() as ctx:
        wpool = ctx.enter_context(tc.tile_pool(name="weights", bufs=1))
        xpool = ctx.enter_context(tc.tile_pool(name="xin", bufs=2))
        ypool = ctx.enter_context(tc.tile_pool(name="yout", bufs=1))
        big = ctx.enter_context(tc.tile_pool(name="big", bufs=2))
        tp_ps = ctx.enter_context(tc.tile_pool(name="tp_ps", bufs=4, space="PSUM"))
        mm_ps = ctx.enter_context(tc.tile_pool(name="mm_ps", bufs=3, space="PSUM"))

        abd = wpool.tile([128, 128], mm_dt)
        nc.sync.dma_start(abd[:], a_h[:])
        gts = wpool.tile([128, 32 * 128], mm_dt)
        nc.sync.dma_start(gts[:], g_h[:])
        bbd = wpool.tile([128, 128], mm_dt)
        nc.sync.dma_start(bbd[:], b_h[:])
        ident = wpool.tile([128, 128], f32)
        make_identity(nc, ident[:])

        # ---- T_in: x (tok,feat) -> xt (feat-chunk partitions, tok cols)
        xt = big.tile([128, 32 * T], mm_dt, tag="big")
        for tt in range(4):
            xs = xpool.tile([128, FEAT], f32, tag="xin")
            nc.sync.dma_start(xs[:], x_h[ts(tt, 128), :])
            for fc in range(32):
                ps = tp_ps.tile([128, 128], f32, tag="tp")
                nc.tensor.transpose(ps[:], xs[:, ts(fc, 128)], ident[:])
                nc.any.tensor_copy(
                    out=xt[:, fc * T + tt * 128: fc * T + tt * 128 + 128],
                    in_=ps[:])

        # ---- S1: rfft
        x1 = big.tile([128, 32 * T], mm_dt, tag="big")
        for fc in range(32):
            ps = mm_ps.tile([128, T], f32, tag="mm")
            nc.tensor.matmul(ps[:], abd[:], xt[:, ts(fc, T)],
                             start=True, stop=True)
            nc.any.tensor_copy(out=x1[:, ts(fc, T)], in_=ps[:])

        # ---- shuffle: X2[64c1+32b+ip, fp*T+t] = X1[64c1+2fp+b, ip*T+t]
        x2 = big.tile([128, 32 * T], mm_dt, tag="big")
        for c1 in range(2):
            for f in range(32):
                for b in range(2):
                    row = 64 * c1 + 2 * f + b
                    src = x1[row: row + 1, :].rearrange("p (i t) -> p i t", t=T)
                    dst = x2[64 * c1 + 32 * b: 64 * c1 + 32 * b + 32, ts(f, T)]
                    nc.sync.dma_start(dst, src)

        # ---- S2: per-freq-pair complex multiply + block contraction
        y2 = big.tile([128, 32 * T], mm_dt, tag="big")
        for fp in range(32):
            ps = mm_ps.tile([128, T], f32, tag="mm")
            nc.tensor.matmul(ps[:], gts[:, ts(fp, 128)], x2[:, ts(fp, T)],
                             start=True, stop=True)
            nc.any.tensor_copy(out=y2[:, ts(fp, T)], in_=ps[:])

        # ---- unshuffle: Y3[64b'+2fp+c1', op*T+t] = Y2[64c1'+32b'+op, fp*T+t]
        y3 = big.tile([128, 32 * T], mm_dt, tag="big")
        for c1 in range(2):
            for b in range(2):
                for f in range(32):
                    row = 64 * b + 2 * f + c1
                    src = y2[64 * c1 + 32 * b: 64 * c1 + 32 * b + 32, ts(f, T)]
                    dst = y3[row: row + 1, :].rearrange("p (o t) -> p o t", t=T)
                    nc.sync.dma_start(dst, src)

        # ---- S3: irfft
        y4 = big.tile([128, 32 * T], f32, tag="big")
        for op in range(32):
            ps = mm_ps.tile([128, T], f32, tag="mm")
            nc.tensor.matmul(ps[:], bbd[:], y3[:, ts(op, T)],
                             start=True, stop=True)
            nc.any.tensor_copy(out=y4[:, ts(op, T)], in_=ps[:])

        # ---- T_out: transpose back to token-major, DMA out
        for tt in range(4):
            ys = ypool.tile([128, FEAT], f32, tag="yout")
            for op in range(32):
                ps = tp_ps.tile([128, 128], f32, tag="tp")
                nc.tensor.transpose(
                    ps[:], y4[:, op * T + tt * 128: op * T + tt * 128 + 128],
                    ident[:])
                nc.any.tensor_copy(out=ys[:, ts(op, 128)], in_=ps[:])
            nc.sync.dma_start(y_h[ts(tt, 128), :], ys[:])

    nc.compile()
    return nc


_CACHE = {}


def kernel(x, circulant_params, channel_weights):
    from concourse.bass_utils import run_bass_kernel_spmd

    x = np.ascontiguousarray(np.asarray(x, np.float32))
    orig_shape = x.shape
    xf = x.reshape(-1, FEAT)
    ntok = xf.shape[0]
    assert ntok == NCORES * T, f"unexpected token count {ntok}"

    A_bd, G, B_bd = _build_matrices(circulant_params, channel_weights)
    g_kfm = np.ascontiguousarray(
        G.transpose(1, 0, 2).reshape(128, 32 * 128).astype(np.float32))

    if "nc" not in _CACHE:
        _CACHE["nc"] = _trace_nc()
    nc = _CACHE["nc"]

    in_maps = [
        {
            "x_shard": np.ascontiguousarray(xf[c * T:(c + 1) * T]),
            "a_bd": A_bd,
            "g_mats": g_kfm,
            "b_bd": B_bd,
        }
        for c in range(NCORES)
    ]
    res = run_bass_kernel_spmd(nc, in_maps, core_ids=list(range(NCORES)))
    y = np.concatenate([res.results[c]["y_shard"] for c in range(NCORES)], axis=0)
    return y.reshape(orig_shape).astype(np.float32)


# revision 7
# speedup vs baseline: 1.4989x; 1.4989x over previous
"""Trainium2 Bass kernel for nn_FFTChainMatrix (block-circulant matmul via 64-pt rFFT).

y = x @ W.T where W is 4096x4096 block-circulant (64x64 grid of 64x64 circulant
blocks) built from channel-weighted circulant_params.  Computed in the FFT
domain as three 128x128-matmul stages per 512-token shard:

  T_in   PE-transpose x (tok-major) -> feature-major
  S1     rfft along block dim:      X1 = A_bd.T @ xt      (per 128-feat chunk)
  shuf   i-pair-major -> freq-pair-major: 32 big SBUF->SBUF DMAs
         X2[:, f*T:+T] <- X1[4f:4f+4, :]   (4-partition rows -> 128-part tile)
  S2     per-freq complex multiply+sum over blocks: Y2 = G[fp].T @ X2
  unshuf inverse: Y3[4f:4f+4, :] <- Y2[:, f*T:+T]
  S3     irfft:                     Y4 = B_bd.T @ Y3
  T_out  PE-transpose back to tok-major, DMA out

Sharding: data-parallel over tokens, 4096 tokens -> 8 cores x 512.
Matmul stages + transposes run as float32r (full-rate fp32 path on the PE).
"""

from contextlib import ExitStack

import numpy as np

BLK = 64
NB = 64           # blocks per side
T = 512           # tokens per core
NCORES = 8
FEAT = 4096

MM_DT = "f32r"    # "f32r" (fast) or "f32" (exact, 4x slower stages)


# ---------------------------------------------------------------- host math
def _build_matrices(circulant_params, channel_weights):
    """A_bd (128,128), G (32,128,128), B_bd (128,128) float32, exact f64 math."""
    c_w = np.einsum(
        "m,moid->oid",
        np.asarray(channel_weights, np.float64),
        np.asarray(circulant_params, np.float64),
    )
    Chat = np.fft.rfft(c_w, axis=-1)
    Wr, Wi = Chat.real, Chat.imag

    r = np.arange(BLK)
    A64 = np.zeros((BLK, BLK))
    A64[0, :] = 1.0
    A64[1, :] = (-1.0) ** r
    B64 = np.zeros((BLK, BLK))
    B64[:, 0] = 1.0 / BLK
    B64[:, 1] = ((-1.0) ** r) / BLK
    for p in range(1, 32):
        cc = np.cos(2 * np.pi * p * r / BLK)
        ss = np.sin(2 * np.pi * p * r / BLK)
        A64[2 * p, :] = cc
        A64[2 * p + 1, :] = -ss
        B64[:, 2 * p] = 2.0 * cc / BLK
        B64[:, 2 * p + 1] = -2.0 * ss / BLK

    # A_bd[k=64b+r, m=4fp+2c1+b] = A64[2fp+c1, r]
    A_bd = np.zeros((128, 128))
    for b in range(2):
        for fp in range(32):
            for c1 in range(2):
                A_bd[64 * b: 64 * b + 64, 4 * fp + 2 * c1 + b] = A64[2 * fp + c1, :]

    # G[fp][k = 64c1 + iperm(i), m = 64c1' + iperm(o)], iperm(i)=32*(i%2)+i//2
    iperm = 32 * (np.arange(NB) % 2) + np.arange(NB) // 2
    G = np.zeros((32, 128, 128))
    for fp in range(32):
        if fp == 0:
            for i in range(NB):
                G[0, iperm[i], iperm] = Wr[:, i, 0]
                G[0, 64 + iperm[i], 64 + iperm] = Wr[:, i, 32]
        else:
            for i in range(NB):
                G[fp, iperm[i], iperm] = Wr[:, i, fp]
                G[fp, 64 + iperm[i], iperm] = -Wi[:, i, fp]
                G[fp, iperm[i], 64 + iperm] = Wi[:, i, fp]
                G[fp, 64 + iperm[i], 64 + iperm] = Wr[:, i, fp]

    # B_bd[k = 4f+2c1+b, m = 64b+r] = B64[r, 2f+c1]
    B_bd = np.zeros((128, 128))
    for f in range(32):
        for c1 in range(2):
            for b in range(2):
                B_bd[4 * f + 2 * c1 + b, 64 * b: 64 * b + 64] = B64[:, 2 * f + c1]

    return (A_bd.astype(np.float32), G.astype(np.float32), B_bd.astype(np.float32))


# ---------------------------------------------------------------- bass trace
def _trace_nc():
    import concourse.mybir as mybir
    import concourse.tile as tile
    from concourse import bacc
    from concourse.bass import ts

    f32 = mybir.dt.float32
    mm_dt = mybir.dt.float32r if MM_DT == "f32r" else mybir.dt.float32

    nc = bacc.Bacc("TRN2", target_bir_lowering=False, debug=False,
                   num_devices=NCORES)
    x_h = nc.dram_tensor("x_shard", [T, FEAT], f32, kind="ExternalInput").ap()
    a_h = nc.dram_tensor("a_bd", [128, 128], mm_dt, kind="ExternalInput").ap()
    g_h = nc.dram_tensor("g_mats", [128, 32 * 128], mm_dt,
                         kind="ExternalInput").ap()
    b_h = nc.dram_tensor("b_bd", [128, 128], mm_dt, kind="ExternalInput").ap()
    i_h = nc.dram_tensor("ident", [128, 128], f32, kind="ExternalInput").ap()
    y_h = nc.dram_tensor("y_shard", [T, FEAT], f32, kind="ExternalOutput").ap()

    copy_ix = [0]

    with tile.TileContext(nc) as tc, ExitStack() as ctx:
        wpool = ctx.enter_context(tc.tile_pool(name="weights", bufs=1))
        xpool = ctx.enter_context(tc.tile_pool(name="xin", bufs=2))
        ypool = ctx.enter_context(tc.tile_pool(name="yout", bufs=1))
        big = ctx.enter_context(tc.tile_pool(name="big", bufs=2))
        tp_ps = ctx.enter_context(tc.tile_pool(name="tp_ps", bufs=4, space="PSUM"))
        mm_ps = ctx.enter_context(tc.tile_pool(name="mm_ps", bufs=3, space="PSUM"))

        def copyback(out_ap, in_ap):
            # 2/3 of copies on DVE, 1/3 on ACT
            if copy_ix[0] % 3 < 2:
                nc.vector.tensor_copy(out_ap, in_ap)
            else:
                nc.scalar.copy(out_ap, in_ap)
            copy_ix[0] += 1

        abd = wpool.tile([128, 128], mm_dt)
        nc.sync.dma_start(abd[:], a_h[:])
        gts = wpool.tile([128, 32 * 128], mm_dt)
        nc.sync.dma_start(gts[:], g_h[:])
        bbd = wpool.tile([128, 128], mm_dt)
        nc.sync.dma_start(bbd[:], b_h[:])
        ident = wpool.tile([128, 128], f32)
        nc.sync.dma_start(ident[:], i_h[:])

        # ---- T_in: x (tok,feat) -> xt (feat-chunk partitions, tok cols)
        xt = big.tile([128, 32 * T], mm_dt, tag="big")
        for tt in range(4):
            xs = xpool.tile([128, FEAT], f32, tag="xin")
            nc.sync.dma_start(xs[:], x_h[ts(tt, 128), :])
            for fc in range(32):
                ps = tp_ps.tile([128, 128], f32, tag="tp")
                nc.tensor.transpose(ps[:], xs[:, ts(fc, 128)], ident[:])
                copyback(xt[:, fc * T + tt * 128: fc * T + tt * 128 + 128],
                         ps[:])

        # ---- S1: rfft
        x1 = big.tile([128, 32 * T], mm_dt, tag="big")
        for fc in range(32):
            ps = mm_ps.tile([128, T], f32, tag="mm")
            nc.tensor.matmul(ps[:], abd[:], xt[:, ts(fc, T)],
                             start=True, stop=True)
            copyback(x1[:, ts(fc, T)], ps[:])

        # ---- shuffle: X2[:, f*T:+T] = X1[4f:4f+4, :]  ((c1,b),ip,t nesting)
        x2 = big.tile([128, 32 * T], mm_dt, tag="big")
        for f in range(32):
            src = x1[4 * f: 4 * f + 4, :].rearrange("p (i t) -> p i t", t=T)
            nc.sync.dma_start(x2[:, ts(f, T)], src)

        # ---- S2: per-freq-pair complex multiply + block contraction
        y2 = big.tile([128, 32 * T], mm_dt, tag="big")
        for fp in range(32):
            ps = mm_ps.tile([128, T], f32, tag="mm")
            nc.tensor.matmul(ps[:], gts[:, ts(fp, 128)], x2[:, ts(fp, T)],
                             start=True, stop=True)
            copyback(y2[:, ts(fp, T)], ps[:])

        # ---- unshuffle: Y3[4f:4f+4, :] = Y2[:, f*T:+T]
        y3 = big.tile([128, 32 * T], mm_dt, tag="big")
        for f in range(32):
            dst = y3[4 * f: 4 * f + 4, :].rearrange("p (o t) -> p o t", t=T)
            nc.sync.dma_start(dst, y2[:, ts(f, T)])

        # ---- S3: irfft
        y4 = big.tile([128, 32 * T], f32, tag="big")
        for op in range(32):
            ps = mm_ps.tile([128, T], f32, tag="mm")
            nc.tensor.matmul(ps[:], bbd[:], y3[:, ts(op, T)],
                             start=True, stop=True)
            copyback(y4[:, ts(op, T)], ps[:])

        # ---- T_out: transpose back to token-major, DMA out
        for tt in range(4):
            ys = ypool.tile([128, FEAT], f32, tag="yout")
            for op in range(32):
                ps = tp_ps.tile([128, 128], f32, tag="tp")
                nc.tensor.transpose(
                    ps[:], y4[:, op * T + tt * 128: op * T + tt * 128 + 128],
                    ident[:])
                copyback(ys[:, ts(op, 128)], ps[:])
            nc.sync.dma_start(y_h[ts(tt, 128), :], ys[:])

    nc.compile()
    return nc


_CACHE = {}


def kernel(x, circulant_params, channel_weights):
    from concourse.bass_utils import run_bass_kernel_spmd

    x = np.ascontiguousarray(np.asarray(x, np.float32))
    orig_shape = x.shape
    xf = x.reshape(-1, FEAT)
    ntok = xf.shape[0]
    assert ntok == NCORES * T, f"unexpected token count {ntok}"

    A_bd, G, B_bd = _build_matrices(circulant_params, channel_weights)
    g_kfm = np.ascontiguousarray(
        G.transpose(1, 0, 2).reshape(128, 32 * 128).astype(np.float32))
    ident = np.eye(128, dtype=np.float32)

    if "nc" not in _CACHE:
        _CACHE["nc"] = _trace_nc()
    nc = _CACHE["nc"]

    in_maps = [
        {
            "x_shard": np.ascontiguousarray(xf[c * T:(c + 1) * T]),
            "a_bd": A_bd,
            "g_mats": g_kfm,
            "b_bd": B_bd,
            "ident": ident,
        }
        for c in range(NCORES)
    ]
    res = run_bass_kernel_spmd(nc, in_maps, core_ids=list(range(NCORES)))
    y = np.concatenate([res.results[c]["y_shard"] for c in range(NCORES)], axis=0)
    return y.reshape(orig_shape).astype(np.float32)


# revision 8
# speedup vs baseline: 1.5710x; 1.0481x over previous
"""Trainium2 Bass kernel for nn_FFTChainMatrix (block-circulant matmul via 64-pt rFFT).

y = x @ W.T where W is 4096x4096 block-circulant (64x64 grid of 64x64 circulant
blocks) built from channel-weighted circulant_params.  Computed in the FFT
domain as three 128x128-matmul stages per 512-token shard:

  T_in   PE-transpose x (tok-major) -> feature-major
  S1     rfft along block dim:      X1 = A_bd.T @ xt      (per 128-feat chunk)
  shuf   i-pair-major -> freq-pair-major: 32 big SBUF->SBUF DMAs
         X2[:, f*T:+T] <- X1[4f:4f+4, :]   (4-partition rows -> 128-part tile)
  S2     per-freq complex multiply+sum over blocks: Y2 = G[fp].T @ X2
  unshuf inverse: Y3[4f:4f+4, :] <- Y2[:, f*T:+T]
  S3     irfft:                     Y4 = B_bd.T @ Y3
  T_out  PE-transpose back to tok-major, DMA out

Sharding: data-parallel over tokens, 4096 tokens -> 8 cores x 512.
Matmul stages + transposes run as float32r (full-rate fp32 path on the PE).
"""

from contextlib import ExitStack

import numpy as np

BLK = 64
NB = 64           # blocks per side
T = 512           # tokens per core
NCORES = 8
FEAT = 4096

MM_DT = "f32r"    # "f32r" (fast) or "f32" (exact, 4x slower stages)


# ---------------------------------------------------------------- host math
def _build_matrices(circulant_params, channel_weights):
    """A_bd (128,128), G (32,128,128), B_bd (128,128) float32, exact f64 math."""
    c_w = np.einsum(
        "m,moid->oid",
        np.asarray(channel_weights, np.float64),
        np.asarray(circulant_params, np.float64),
    )
    Chat = np.fft.rfft(c_w, axis=-1)
    Wr, Wi = Chat.real, Chat.imag

    r = np.arange(BLK)
    A64 = np.zeros((BLK, BLK))
    A64[0, :] = 1.0
    A64[1, :] = (-1.0) ** r
    B64 = np.zeros((BLK, BLK))
    B64[:, 0] = 1.0 / BLK
    B64[:, 1] = ((-1.0) ** r) / BLK
    for p in range(1, 32):
        cc = np.cos(2 * np.pi * p * r / BLK)
        ss = np.sin(2 * np.pi * p * r / BLK)
        A64[2 * p, :] = cc
        A64[2 * p + 1, :] = -ss
        B64[:, 2 * p] = 2.0 * cc / BLK
        B64[:, 2 * p + 1] = -2.0 * ss / BLK

    # A_bd[k=64b+r, m=4fp+2c1+b] = A64[2fp+c1, r]
    A_bd = np.zeros((128, 128))
    for b in range(2):
        for fp in range(32):
            for c1 in range(2):
                A_bd[64 * b: 64 * b + 64, 4 * fp + 2 * c1 + b] = A64[2 * fp + c1, :]

    # G[fp][k = 64c1 + iperm(i), m = 64c1' + iperm(o)], iperm(i)=32*(i%2)+i//2
    iperm = 32 * (np.arange(NB) % 2) + np.arange(NB) // 2
    G = np.zeros((32, 128, 128))
    for fp in range(32):
        if fp == 0:
            for i in range(NB):
                G[0, iperm[i], iperm] = Wr[:, i, 0]
                G[0, 64 + iperm[i], 64 + iperm] = Wr[:, i, 32]
        else:
            for i in range(NB):
                G[fp, iperm[i], iperm] = Wr[:, i, fp]
                G[fp, 64 + iperm[i], iperm] = -Wi[:, i, fp]
                G[fp, iperm[i], 64 + iperm] = Wi[:, i, fp]
                G[fp, 64 + iperm[i], 64 + iperm] = Wr[:, i, fp]

    # B_bd[k = 4f+2c1+b, m = 64b+r] = B64[r, 2f+c1]
    B_bd = np.zeros((128, 128))
    for f in range(32):
        for c1 in range(2):
            for b in range(2):
                B_bd[4 * f + 2 * c1 + b, 64 * b: 64 * b + 64] = B64[:, 2 * f + c1]

    return (A_bd.astype(np.float32), G.astype(np.float32), B_bd.astype(np.float32))


# ---------------------------------------------------------------- bass trace
def _trace_nc():
    import concourse.mybir as mybir
    import concourse.tile as tile
    from concourse import bacc
    from concourse.bass import ts

    f32 = mybir.dt.float32
    mm_dt = mybir.dt.float32r if MM_DT == "f32r" else mybir.dt.float32

    nc = bacc.Bacc("TRN2", target_bir_lowering=False, debug=False,
                   num_devices=NCORES)
    x_h = nc.dram_tensor("x_shard", [T, FEAT], f32, kind="ExternalInput").ap()
    a_h = nc.dram_tensor("a_bd", [128, 128], mm_dt, kind="ExternalInput").ap()
    g_h = nc.dram_tensor("g_mats", [128, 32 * 128], mm_dt,
                         kind="ExternalInput").ap()
    b_h = nc.dram_tensor("b_bd", [128, 128], mm_dt, kind="ExternalInput").ap()
    i_h = nc.dram_tensor("ident", [128, 128], f32, kind="ExternalInput").ap()
    y_h = nc.dram_tensor("y_shard", [T, FEAT], f32, kind="ExternalOutput").ap()

    copy_ix = [0]

    with tile.TileContext(nc) as tc, ExitStack() as ctx:
        wpool = ctx.enter_context(tc.tile_pool(name="weights", bufs=1))
        xpool = ctx.enter_context(tc.tile_pool(name="xin", bufs=2))
        ypool = ctx.enter_context(tc.tile_pool(name="yout", bufs=1))
        big = ctx.enter_context(tc.tile_pool(name="big", bufs=2))
        tp_ps = ctx.enter_context(tc.tile_pool(name="tp_ps", bufs=4, space="PSUM"))
        mm_ps = ctx.enter_context(tc.tile_pool(name="mm_ps", bufs=3, space="PSUM"))

        def copyback(out_ap, in_ap):
            # 2/3 of copies on DVE, 1/3 on ACT
            if copy_ix[0] % 3 < 2:
                nc.vector.tensor_copy(out_ap, in_ap)
            else:
                nc.scalar.copy(out_ap, in_ap)
            copy_ix[0] += 1

        abd = wpool.tile([128, 128], mm_dt)
        nc.sync.dma_start(abd[:], a_h[:])
        gts = wpool.tile([128, 32 * 128], mm_dt)
        nc.sync.dma_start(gts[:], g_h[:])
        bbd = wpool.tile([128, 128], mm_dt)
        nc.sync.dma_start(bbd[:], b_h[:])
        ident = wpool.tile([128, 128], f32)
        nc.sync.dma_start(ident[:], i_h[:])

        # ---- T_in: x (tok,feat) -> xt (feat-chunk partitions, tok cols)
        xt = big.tile([128, 32 * T], mm_dt, tag="big")
        for tt in range(4):
            xs = xpool.tile([128, FEAT], f32, tag="xin")
            (nc.sync if tt % 2 == 0 else nc.scalar).dma_start(
                xs[:], x_h[ts(tt, 128), :])
            for fc in range(32):
                ps = tp_ps.tile([128, 128], f32, tag="tp")
                nc.tensor.transpose(ps[:], xs[:, ts(fc, 128)], ident[:])
                copyback(xt[:, fc * T + tt * 128: fc * T + tt * 128 + 128],
                         ps[:])

        # ---- S1: rfft
        x1 = big.tile([128, 32 * T], mm_dt, tag="big")
        for fc in range(32):
            ps = mm_ps.tile([128, T], f32, tag="mm")
            nc.tensor.matmul(ps[:], abd[:], xt[:, ts(fc, T)],
                             start=True, stop=True)
            copyback(x1[:, ts(fc, T)], ps[:])

        # ---- shuffle: X2[:, f*T:+T] = X1[4f:4f+4, :]  ((c1,b),ip,t nesting)
        x2 = big.tile([128, 32 * T], mm_dt, tag="big")
        for f in range(32):
            eng = (nc.sync, nc.scalar, nc.gpsimd)[f % 3]
            src_ap = x1[4 * f: 4 * f + 4, :].rearrange("p (i t) -> p i t", t=T)
            eng.dma_start(x2[:, ts(f, T)], src_ap)

        # ---- S2: per-freq-pair complex multiply + block contraction
        y2 = big.tile([128, 32 * T], mm_dt, tag="big")
        for fp in range(32):
            ps = mm_ps.tile([128, T], f32, tag="mm")
            nc.tensor.matmul(ps[:], gts[:, ts(fp, 128)], x2[:, ts(fp, T)],
                             start=True, stop=True)
            copyback(y2[:, ts(fp, T)], ps[:])

        # ---- unshuffle: Y3[4f:4f+4, :] = Y2[:, f*T:+T]
        y3 = big.tile([128, 32 * T], mm_dt, tag="big")
        for f in range(32):
            eng = (nc.sync, nc.scalar, nc.gpsimd)[f % 3]
            dst = y3[4 * f: 4 * f + 4, :].rearrange("p (o t) -> p o t", t=T)
            eng.dma_start(dst, y2[:, ts(f, T)])

        # ---- S3: irfft
        y4 = big.tile([128, 32 * T], f32, tag="big")
        for op in range(32):
            ps = mm_ps.tile([128, T], f32, tag="mm")
            nc.tensor.matmul(ps[:], bbd[:], y3[:, ts(op, T)],
                             start=True, stop=True)
            copyback(y4[:, ts(op, T)], ps[:])

        # ---- T_out: transpose back to token-major, DMA out
        for tt in range(4):
            ys = ypool.tile([128, FEAT], f32, tag="yout")
            for op in range(32):
                ps = tp_ps.tile([128, 128], f32, tag="tp")
                nc.tensor.transpose(
                    ps[:], y4[:, op * T + tt * 128: op * T + tt * 128 + 128],
                    ident[:])
                copyback(ys[:, ts(op, 128)], ps[:])
            (nc.sync if tt % 2 == 0 else nc.scalar).dma_start(
                y_h[ts(tt, 128), :], ys[:])

    nc.compile()
    return nc


_CACHE = {}


def kernel(x, circulant_params, channel_weights):
    from concourse.bass_utils import run_bass_kernel_spmd

    x = np.ascontiguousarray(np.asarray(x, np.float32))
    orig_shape = x.shape
    xf = x.reshape(-1, FEAT)
    ntok = xf.shape[0]
    assert ntok == NCORES * T, f"unexpected token count {ntok}"

    A_bd, G, B_bd = _build_matrices(circulant_params, channel_weights)
    g_kfm = np.ascontiguousarray(
        G.transpose(1, 0, 2).reshape(128, 32 * 128).astype(np.float32))
    ident = np.eye(128, dtype=np.float32)

    if "nc" not in _CACHE:
        _CACHE["nc"] = _trace_nc()
    nc = _CACHE["nc"]

    in_maps = [
        {
            "x_shard": np.ascontiguousarray(xf[c * T:(c + 1) * T]),
            "a_bd": A_bd,
            "g_mats": g_kfm,
            "b_bd": B_bd,
            "ident": ident,
        }
        for c in range(NCORES)
    ]
    res = run_bass_kernel_spmd(nc, in_maps, core_ids=list(range(NCORES)))
    y = np.concatenate([res.results[c]["y_shard"] for c in range(NCORES)], axis=0)
    return y.reshape(orig_shape).astype(np.float32)
